# revision 1
# baseline (speedup 1.0000x reference)
"""MoE expert-parallel kernel for Trainium2 (8 NeuronCores).

Problem: nn_DistributedExpertPool — each of 2048 tokens (H=1024) is routed to
one of 8 experts; expert e applies Linear(H->F=2048) -> exact GELU ->
Linear(F->H).

Strategy (expert parallelism, matching the sharding hint):
  - Host: sort tokens by expert assignment ("dispatch"), pad each expert's
    token batch to a common capacity CAP (multiple of 128), and pre-transpose
    to x.T layout [H, CAP] so the device kernel only ever streams K-major
    operands.
  - Core c gets expert c's weights (W1[c] [H,F], W2[c] [F,H], biases) plus its
    token batch. Device computes y.T = W2.T @ gelu(W1.T @ x.T + b1) + b2
    entirely on-chip (weights resident in SBUF, PSUM accumulation over K).
  - Host: scatter each core's outputs back to the original token order
    ("combine").

The device kernel keeps both matmuls in the transposed layout so the GELU
bias (b1, per-F) and the output bias (b2, per-H) are per-partition vectors,
which the ScalarE activation op applies for free.

Matmul operands stream as fp16 (weights are ~N(0, 0.02), activations O(1) —
well inside fp16 range; 10-bit mantissa beats bf16 by 8x here). PSUM
accumulation stays fp32 and the output is stored fp32. Measured 4.2e-4
relative error end-to-end vs the fp32 reference. KM_MMDT=fp32r (2.1e-4,
~1.5x slower) and KM_MMDT=fp32 (4e-7, ~5x slower) are exactness fallbacks.
"""

import os as _os
import sys as _sys

import numpy as np

try:
    import concourse.bass as bass
except ImportError:  # fresh dirs without the site hook on sys.path
    for _p in ("/opt/trn_rl_repo", "/root/.axon_site/_ro/trn_rl_repo"):
        if _p not in _sys.path:
            _sys.path.append(_p)
    import concourse.bass as bass  # noqa: E402
import concourse.tile as tile
from concourse import mybir
from concourse.bass_utils import run_bass_kernel_spmd  # noqa: F401 (fallback)

_jit_cache: dict[int, tuple] = {}


def _run_spmd_cached(nc, in_maps):
    """run_bass_kernel_spmd's axon/PJRT path with the jitted executable cached
    per program — the concourse shim rebuilds its jax.jit closure every call,
    paying ~1.5s of retrace; reusing one function object makes repeat calls
    dispatch in milliseconds."""
    import jax
    import numpy as _np
    from jax.sharding import Mesh, PartitionSpec
    from jax.experimental.shard_map import shard_map
    from concourse import bass2jax, mybir as _mb

    key = id(nc)
    if key not in _jit_cache:
        bass2jax.install_neuronx_cc_hook()
        partition_name = (nc.partition_id_tensor.name
                          if nc.partition_id_tensor else None)
        in_names, out_names, out_avals = [], [], []
        for alloc in nc.m.functions[0].allocations:
            if not isinstance(alloc, _mb.MemoryLocationSet):
                continue
            name = alloc.memorylocations[0].name
            if alloc.kind == "ExternalInput":
                if name != partition_name:
                    in_names.append(name)
            elif alloc.kind == "ExternalOutput":
                out_names.append(name)
                out_avals.append(jax.core.ShapedArray(
                    tuple(alloc.tensor_shape), _mb.dt.np(alloc.dtype)))
        n_params = len(in_names)
        all_names = list(in_names) + list(out_names)
        if partition_name is not None:
            all_names.append(partition_name)

        def _body(*args):
            operands = list(args)
            if partition_name is not None:
                operands.append(bass2jax.partition_id_tensor())
            return tuple(bass2jax._bass_exec_p.bind(
                *operands, out_avals=tuple(out_avals),
                in_names=tuple(all_names), out_names=tuple(out_names),
                lowering_input_output_aliases=(),
                sim_require_finite=True, sim_require_nnan=True, nc=nc))

        devices = jax.devices()[:N_CORES]
        mesh = Mesh(_np.asarray(devices), ("core",))
        n_outs = len(out_names)
        sharded = jax.jit(
            shard_map(_body, mesh=mesh,
                      in_specs=(PartitionSpec("core"),) * (n_params + n_outs),
                      out_specs=(PartitionSpec("core"),) * n_outs,
                      check_rep=False),
            donate_argnums=tuple(range(n_params, n_params + n_outs)),
            keep_unused=True)
        _jit_cache[key] = (sharded, in_names, out_names, out_avals, n_params)

    sharded, in_names, out_names, out_avals, n_params = _jit_cache[key]
    concat_in = [
        _np.concatenate([_np.asarray(m[name]) for m in in_maps], axis=0)
        for name in in_names]
    concat_zeros = [
        _np.zeros((N_CORES * a.shape[0], *a.shape[1:]), a.dtype)
        for a in out_avals]
    out_arrs = sharded(*concat_in, *concat_zeros)

    class _R:
        results = [
            {name: _np.asarray(out_arrs[i]).reshape(
                N_CORES, *out_avals[i].shape)[c]
             for i, name in enumerate(out_names)}
            for c in range(N_CORES)]
    return _R()

TOKENS = 2048
HIDDEN = 1024
FFN = 2048
NUM_EXPERTS = 8
N_CORES = 8

KH = HIDDEN // 128  # 8 K-tiles for the first matmul
KF = FFN // 128     # 16 K-tiles for the second matmul

_compiled_cache: dict[tuple, bass.Bass] = {}

# PE streaming dtype for matmul operands: float32 = exact two-pass (4 cyc/row),
# float32r = single-pass reduced-precision (1 cyc/row at N>=256).
MM_DTYPE = {"fp32": mybir.dt.float32, "fp32r": mybir.dt.float32r,
            "fp16": mybir.dt.float16}[_os.environ.get("KM_MMDT", "fp16")]


def _split_multi_waits(nc: bass.Bass) -> None:
    """Walrus in this toolchain accepts at most ONE sync-wait per instruction
    ("Too many sync wait commands" in setupSyncWait otherwise). Tile's
    scheduler happily attaches several. Split the extras into NoOps placed
    just before the instruction on the same engine queue — the NX sequencer
    processes them in order, so the semantics are identical."""
    for fn in nc.m.functions:
        for blk in fn.blocks:
            out = []
            changed = False
            for inst in blk.instructions:
                si = inst.sync_info
                if si is not None and si.on_wait is not None and len(si.on_wait) > 1:
                    waits = list(si.on_wait)
                    for j, w in enumerate(waits[:-1]):
                        nop = mybir.InstNoOp(
                            name=f"{inst.name}-wsplit{j}", ins=[], outs=[])
                        nop.engine = inst.engine
                        nop.sync_info = mybir.SyncInfo(on_wait=[w], on_update=[])
                        out.append(nop)
                    inst.sync_info = mybir.SyncInfo(
                        on_wait=[waits[-1]],
                        on_update=list(si.on_update) if si.on_update else [],
                    )
                    changed = True
                out.append(inst)
            if changed:
                blk.instructions = out


def _build_nc(cap: int, mm_dtype=None) -> bass.Bass:
    """Build the per-core Bass program for token capacity `cap` (mult of 128)."""
    fp32 = mybir.dt.float32
    mmdt = MM_DTYPE if mm_dtype is None else mm_dtype
    nc = bass.Bass("TRN2", target_bir_lowering=False, debug=False,
                   num_devices=N_CORES)

    # All streaming operands are host-preswizzled into per-partition
    # contiguous images, so every device DMA is a plain 2-D slice copy:
    #   xs0: all of x.T + W1's first single strip (smallest possible first
    #        chunk -> earliest PE start)
    #   w1s: W1's second single strip, then strip pairs 1..7
    #   w2s: W2 in quad-chunk layout [p][quad][k][h]
    xs0 = nc.dram_tensor("xs0", [128, KH * cap + KH * 128], mmdt,
                         kind="ExternalInput").ap()
    w1s_d = nc.dram_tensor("w1s", [128, KH * 128 + 7 * KH * 256], mmdt,
                           kind="ExternalInput").ap()
    w2s_d = nc.dram_tensor("w2s", [128, KF * HIDDEN], mmdt,
                           kind="ExternalInput").ap()
    # biases pre-swizzled on host to [128, KF] / [128, KH] (partition-major)
    b1 = nc.dram_tensor("b1", [128, KF], fp32, kind="ExternalInput").ap()
    b2 = nc.dram_tensor("b2", [128, KH], fp32, kind="ExternalInput").ap()
    yT = nc.dram_tensor("yT", [HIDDEN, cap], fp32, kind="ExternalOutput").ap()

    # Phase-1 weights stream as M-strip pairs (all K rows for two 128-wide F
    # tiles, >=512B contiguous runs per partition): a strip's matmuls finish
    # one PSUM bank, the GELU drains it, and the bank recycles — the PE tracks
    # the DMA stream with a few live banks instead of needing all 16
    # accumulators at once.

    with tile.TileContext(nc) as tc:
        with (
            tc.tile_pool(name="xt_pool", bufs=KH) as xt_pool,
            tc.tile_pool(name="w1_pool", bufs=8) as w1_pool,
            tc.tile_pool(name="w2_pool", bufs=1) as w2_pool,
            tc.tile_pool(name="bias_pool", bufs=1) as bias_pool,
            tc.tile_pool(name="ht_pool", bufs=KF) as ht_pool,
            tc.tile_pool(name="out_pool", bufs=4) as out_pool,
            tc.tile_pool(name="ps_pool", bufs=8, space="PSUM") as ps_pool,
        ):
            # x.T + W1's first single strip in ONE contiguous DMA
            xw0 = xt_pool.tile([128, KH * cap + KH * 128], mmdt, name="xw0",
                               tag="xw0", bufs=1)
            nc.sync.dma_start(xw0[:], xs0[:])
            xta = xw0[:, :KH * cap]
            strip0a = xw0[:, KH * cap:]
            strip0b = w1_pool.tile([128, KH * 128], mmdt, name="w1s0b",
                                   tag="w1s0b", bufs=1)
            nc.sync.dma_start(strip0b[:], w1s_d[:, :KH * 128])

            def load_w1_strip_pair(mp):
                t = w1_pool.tile([128, KH * 256], mmdt, name=f"w1s{mp}",
                                 tag="w1s")
                off = KH * 128 + (mp - 1) * KH * 256
                nc.sync.dma_start(t[:], w1s_d[:, off:off + KH * 256])
                return t

            # PE p-state warmup: the PE runs at half clock until ~3us of
            # continuous busy (HAM). Burn the DMA-wait window with tiny
            # matmuls on a zeroed tile so the real stream starts warm.
            wz = bias_pool.tile([128, 128], mmdt, name="wz", tag="wz")
            nc.gpsimd.memset(wz[:], 0.0)
            wps = ps_pool.tile([128, 128], fp32, name="wps", tag="ps")
            for i in range(40):
                nc.tensor.matmul(wps[:], wz[:], wz[:],
                                 start=True, stop=True)

            # ---- phase 1: hT[m] = gelu(W1.T @ xT + b1)  [F on partitions] ----
            hts = [None] * KF
            for mp in range(KF // 2):
                strip = None if mp == 0 else load_w1_strip_pair(mp)
                if mp == 0:
                    b1s = bias_pool.tile([128, KF], fp32, name="b1s", tag="b1s")
                    nc.scalar.dma_start(b1s[:], b1[:])
                    b2s = bias_pool.tile([128, KH], fp32, name="b2s", tag="b2s")
                    nc.scalar.dma_start(b2s[:], b2[:])
                for half in range(2):
                    m = 2 * mp + half
                    psb = ps_pool.tile([128, cap], fp32, name=f"ps1_{m}",
                                       tag="ps")
                    for k in range(KH):
                        if mp == 0:
                            src = strip0a if half == 0 else strip0b
                            lhsT = src[:, k * 128:(k + 1) * 128]
                        else:
                            off = k * 256 + half * 128
                            lhsT = strip[:, off:off + 128]
                        nc.tensor.matmul(
                            psb[:], lhsT,
                            xta[:, k * cap:(k + 1) * cap],
                            start=(k == 0), stop=(k == KH - 1))
                    ht = ht_pool.tile([128, cap], mmdt, name=f"ht{m}",
                                      tag="ht")
                    nc.scalar.activation(
                        ht[:], psb[:],
                        mybir.ActivationFunctionType.Gelu,
                        bias=b1s[:, m:m + 1])
                    hts[m] = ht

            # ---- phase 2: yT[m] = W2.T @ hT + b2  [H on partitions] ----
            # W2 streams in 2 MB quads during phase 1 and is fully resident
            # well before the PE reaches it. Each m-chain then runs its 16
            # matmuls back-to-back and evacuates immediately — stores stream
            # inline with the remaining chains instead of bunching at the end.
            w2qs = []
            for q in range(4):
                t = w2_pool.tile([128, 4 * HIDDEN], mmdt, name=f"w2q{q}",
                                 tag=f"w2q{q}", bufs=1)
                nc.sync.dma_start(
                    t[:], w2s_d[:, q * 4 * HIDDEN:(q + 1) * 4 * HIDDEN])
                w2qs.append(t)

            def w2_lhsT(k, m):
                q, kk = divmod(k, 4)
                off = kk * HIDDEN + m * 128
                return w2qs[q][:, off:off + 128]

            ps2 = [ps_pool.tile([128, cap], fp32, name=f"ps2_{m}", tag="ps")
                   for m in range(KH)]
            ot = None
            for m in range(KH):
                for k in range(KF):
                    nc.tensor.matmul(
                        ps2[m][:], w2_lhsT(k, m), hts[k][:],
                        start=(k == 0), stop=(k == KF - 1))
                # bias-add into an m-pair tile (DVE even m, ACT odd m) and
                # store both halves in one DMA on alternating rings
                if m >= KH - 2:
                    # final chains: single-m stores, shortest possible tail
                    os_ = out_pool.tile([128, cap], fp32,
                                        name=f"os{m}", tag=f"os{m % 2}")
                    nc.vector.tensor_scalar_add(
                        os_[:], ps2[m][:], b2s[:, m:m + 1])
                    eng = nc.scalar if m % 2 == 0 else nc.sync
                    eng.dma_start(yT[m * 128:(m + 1) * 128, :], os_[:])
                elif m % 2 == 0:
                    ot = out_pool.tile([128, 2 * cap], fp32,
                                       name=f"ot{m}", tag="ot")
                    nc.vector.tensor_scalar_add(
                        ot[:, :cap], ps2[m][:], b2s[:, m:m + 1])
                else:
                    nc.scalar.activation(
                        ot[:, cap:], ps2[m][:],
                        mybir.ActivationFunctionType.Identity,
                        bias=b2s[:, m:m + 1])
                    eng = nc.scalar if (m // 2) % 2 == 0 else nc.sync
                    eng.dma_start(
                        yT[(m - 1) * 128:(m + 1) * 128, :]
                        .rearrange("(c p) t -> p c t", p=128),
                        ot.rearrange("p (c t) -> p c t", c=2))

    _split_multi_waits(nc)
    return nc


def _get_nc(cap: int) -> bass.Bass:
    key = (cap, MM_DTYPE)
    if key not in _compiled_cache:
        _compiled_cache[key] = _build_nc(cap, MM_DTYPE)
    return _compiled_cache[key]


def _reference_numpy(x, idx, W1, b1, W2, b2):
    """Exact CPU path (erf-gelu in float64). Used only if routing is so
    imbalanced that one expert exceeds 512 tokens (breaks the device tiling)
    or the device path fails — slow but correct."""
    import math
    erf = np.vectorize(math.erf, otypes=[np.float64])
    out = np.zeros_like(x, dtype=np.float64)
    for e in range(NUM_EXPERTS):
        rows = np.nonzero(idx == e)[0]
        if rows.size == 0:
            continue
        h = x[rows].astype(np.float64) @ W1[e].astype(np.float64) + b1[e]
        h = h * 0.5 * (1.0 + erf(h / np.sqrt(2.0)))
        out[rows] = h @ W2[e].astype(np.float64) + b2[e]
    return out.astype(np.float32)


def kernel(x, expert_indices, W1, b1, W2, b2):
    x = np.ascontiguousarray(np.asarray(x, dtype=np.float32))
    idx = np.asarray(expert_indices).astype(np.int64)
    W1 = np.asarray(W1, dtype=np.float32)
    W2 = np.asarray(W2, dtype=np.float32)
    b1 = np.asarray(b1, dtype=np.float32)
    b2 = np.asarray(b2, dtype=np.float32)

    counts = np.bincount(idx, minlength=NUM_EXPERTS)
    # fp32r single-pass mode needs moving dim >= 256; one PSUM bank caps it
    # at 512 fp32. Even-align for 4B-multiple rows in every dtype.
    cap = max(256, int(-(-int(counts.max()) // 2)) * 2)
    if cap > 512:  # pathological routing, exceeds one PSUM bank
        return _reference_numpy(x, idx, W1, b1, W2, b2)
    nc = _get_nc(cap)

    # dispatch: stable sort tokens by expert
    order = np.argsort(idx, kind="stable")
    starts = np.zeros(NUM_EXPERTS + 1, dtype=np.int64)
    np.cumsum(counts, out=starts[1:])

    np_mmdt = np.float16 if MM_DTYPE == mybir.dt.float16 else np.float32
    in_maps = []
    tok_of_core = []
    for e in range(NUM_EXPERTS):
        toks = order[starts[e]:starts[e + 1]]
        tok_of_core.append(toks)
        xs = np.zeros((HIDDEN, cap), dtype=np_mmdt)
        xs[:, :len(toks)] = x[toks].T
        w1e = W1[e].astype(np_mmdt)
        xs0 = np.concatenate([
            xs.reshape(KH, 128, cap).transpose(1, 0, 2).reshape(128, -1),
            w1e[:, :128].reshape(KH, 128, 128)
            .transpose(1, 0, 2).reshape(128, -1),
        ], axis=1)
        w1s = np.concatenate([
            w1e[:, 128:256].reshape(KH, 128, 128)
            .transpose(1, 0, 2).reshape(128, -1),
            w1e[:, 256:].reshape(KH, 128, 7, 256)
            .transpose(1, 2, 0, 3).reshape(128, -1),
        ], axis=1)
        w2s = (W2[e].astype(np_mmdt).reshape(4, 4, 128, HIDDEN)
               .transpose(2, 0, 1, 3).reshape(128, -1))
        in_maps.append({
            "xs0": np.ascontiguousarray(xs0),
            "w1s": np.ascontiguousarray(w1s),
            "w2s": np.ascontiguousarray(w2s),
            "b1": np.ascontiguousarray(b1[e].reshape(KF, 128).T),
            "b2": np.ascontiguousarray(b2[e].reshape(KH, 128).T),
        })

    try:
        res = _run_spmd_cached(nc, in_maps)
    except Exception:
        try:  # transient failures recover on retry; fall back to the shim
            res = run_bass_kernel_spmd(nc, in_maps,
                                       core_ids=list(range(N_CORES)))
        except Exception:
            return _reference_numpy(x, idx, W1, b1, W2, b2)
    global LAST_RESULTS
    LAST_RESULTS = res

    out = np.zeros((TOKENS, HIDDEN), dtype=np.float32)
    for e in range(NUM_EXPERTS):
        toks = tok_of_core[e]
        out[toks] = res.results[e]["yT"][:, :len(toks)].T
    return out



# revision 3
# speedup vs baseline: 1.0497x; 1.0497x over previous
"""MoE expert-parallel kernel for Trainium2 (8 NeuronCores).

Problem: nn_DistributedExpertPool — each of 2048 tokens (H=1024) is routed to
one of 8 experts; expert e applies Linear(H->F=2048) -> exact GELU ->
Linear(F->H).

Strategy (expert parallelism, matching the sharding hint):
  - Host: sort tokens by expert assignment ("dispatch"), pad each expert's
    token batch to a common capacity CAP (multiple of 2), and pre-swizzle all
    streaming operands into per-partition contiguous images so every device
    DMA is a plain 2-D slice copy.
  - Core c gets expert c's weights plus its token batch and computes
    y.T = W2.T @ gelu(W1.T @ x.T + b1) + b2 entirely on-chip.
  - Host: scatter each core's outputs back to the original token order
    ("combine").

Schedule (everything is paced by the single 360 GB/s DMA pipe, so the stream
order is the design):
  - Phase 1 runs its first 8 F-columns K-MAJOR over eight "slabs"
    [x_k | W1_k,m0..7] whose ~920ns delivery cadence exactly matches the
    912ns PE sweep, so the PE starts ~4.2us in (DMA preamble + first slab +
    semaphore latency is the hard floor) and never starves. b1 rides in
    slab0 as 16 fp16 columns. The remaining 8 F-columns run as m-chains on
    256KB strips (728ns delivery vs 912ns consumption).
  - Phase 2 streams W2 as 16 k-slabs behind the strips and runs k-major
    sweeps for k=0..7 (PSUM-bank bound: exactly 8 accumulators), then
    finishes each m-chain with k=8..15 so chains complete 912ns apart and
    their bias-add + store pipelines hide behind compute; only the last
    chain's evacuate/store/semaphore tail (~3.7us, dominated by fixed DGE
    descgen 625 + trigger delay 650 + completion semaphore 900ns) is
    exposed.
  - Stores are fp16 (host upcasts); output rel-err stays ~5e-4.
  - Two 1-row warmup matmuls pin the PE p-state ramp start early; the first
    two real matmuls still pay the mid-p-state rate, everything after runs
    at full clock.

Matmul operands stream as fp16 (weights are ~N(0, 0.02), activations O(1) —
well inside fp16 range). PSUM accumulation stays fp32. Measured ~5e-4
relative error end-to-end vs the fp32 reference.
"""

import os as _os
import sys as _sys

import numpy as np

try:
    import concourse.bass as bass
except ImportError:  # fresh dirs without the site hook on sys.path
    for _p in ("/opt/trn_rl_repo", "/root/.axon_site/_ro/trn_rl_repo"):
        if _p not in _sys.path:
            _sys.path.append(_p)
    import concourse.bass as bass  # noqa: E402
import concourse.tile as tile
from concourse import mybir
from concourse.bass_utils import run_bass_kernel_spmd  # noqa: F401 (fallback)

_jit_cache: dict[int, tuple] = {}


def _run_spmd_cached(nc, in_maps):
    """run_bass_kernel_spmd's axon/PJRT path with the jitted executable cached
    per program — the concourse shim rebuilds its jax.jit closure every call,
    paying ~1.5s of retrace; reusing one function object makes repeat calls
    dispatch in milliseconds."""
    import jax
    import numpy as _np
    from jax.sharding import Mesh, PartitionSpec
    from jax.experimental.shard_map import shard_map
    from concourse import bass2jax, mybir as _mb

    key = id(nc)
    if key not in _jit_cache:
        bass2jax.install_neuronx_cc_hook()
        partition_name = (nc.partition_id_tensor.name
                          if nc.partition_id_tensor else None)
        in_names, out_names, out_avals = [], [], []
        for alloc in nc.m.functions[0].allocations:
            if not isinstance(alloc, _mb.MemoryLocationSet):
                continue
            name = alloc.memorylocations[0].name
            if alloc.kind == "ExternalInput":
                if name != partition_name:
                    in_names.append(name)
            elif alloc.kind == "ExternalOutput":
                out_names.append(name)
                out_avals.append(jax.core.ShapedArray(
                    tuple(alloc.tensor_shape), _mb.dt.np(alloc.dtype)))
        n_params = len(in_names)
        all_names = list(in_names) + list(out_names)
        if partition_name is not None:
            all_names.append(partition_name)

        def _body(*args):
            operands = list(args)
            if partition_name is not None:
                operands.append(bass2jax.partition_id_tensor())
            return tuple(bass2jax._bass_exec_p.bind(
                *operands, out_avals=tuple(out_avals),
                in_names=tuple(all_names), out_names=tuple(out_names),
                lowering_input_output_aliases=(),
                sim_require_finite=True, sim_require_nnan=True, nc=nc))

        devices = jax.devices()[:N_CORES]
        mesh = Mesh(_np.asarray(devices), ("core",))
        n_outs = len(out_names)
        sharded = jax.jit(
            shard_map(_body, mesh=mesh,
                      in_specs=(PartitionSpec("core"),) * (n_params + n_outs),
                      out_specs=(PartitionSpec("core"),) * n_outs,
                      check_rep=False),
            donate_argnums=tuple(range(n_params, n_params + n_outs)),
            keep_unused=True)
        _jit_cache[key] = (sharded, in_names, out_names, out_avals, n_params)

    sharded, in_names, out_names, out_avals, n_params = _jit_cache[key]
    concat_in = [
        _np.concatenate([_np.asarray(m[name]) for m in in_maps], axis=0)
        for name in in_names]
    concat_zeros = [
        _np.zeros((N_CORES * a.shape[0], *a.shape[1:]), a.dtype)
        for a in out_avals]
    out_arrs = sharded(*concat_in, *concat_zeros)

    class _R:
        results = [
            {name: _np.asarray(out_arrs[i]).reshape(
                N_CORES, *out_avals[i].shape)[c]
             for i, name in enumerate(out_names)}
            for c in range(N_CORES)]
    return _R()

TOKENS = 2048
HIDDEN = 1024
FFN = 2048
NUM_EXPERTS = 8
N_CORES = 8

KH = HIDDEN // 128  # 8 K-tiles for the first matmul
KF = FFN // 128     # 16 K-tiles for the second matmul
KBREAK = 8          # phase 2: k-major sweeps below, m-chains above

_compiled_cache: dict[tuple, bass.Bass] = {}

MM_DTYPE = {"fp32": mybir.dt.float32, "fp32r": mybir.dt.float32r,
            "fp16": mybir.dt.float16}[_os.environ.get("KM_MMDT", "fp16")]


def _split_multi_waits(nc: bass.Bass) -> None:
    """Walrus in this toolchain accepts at most ONE sync-wait per instruction
    ("Too many sync wait commands" in setupSyncWait otherwise). Tile's
    scheduler happily attaches several. Split the extras into NoOps placed
    just before the instruction on the same engine queue — the NX sequencer
    processes them in order, so the semantics are identical."""
    for fn in nc.m.functions:
        for blk in fn.blocks:
            out = []
            changed = False
            for inst in blk.instructions:
                si = inst.sync_info
                if si is not None and si.on_wait is not None and len(si.on_wait) > 1:
                    waits = list(si.on_wait)
                    for j, w in enumerate(waits[:-1]):
                        nop = mybir.InstNoOp(
                            name=f"{inst.name}-wsplit{j}", ins=[], outs=[])
                        nop.engine = inst.engine
                        nop.sync_info = mybir.SyncInfo(on_wait=[w], on_update=[])
                        out.append(nop)
                    inst.sync_info = mybir.SyncInfo(
                        on_wait=[waits[-1]],
                        on_update=list(si.on_update) if si.on_update else [],
                    )
                    changed = True
                out.append(inst)
            if changed:
                blk.instructions = out


def _build_nc(cap: int, mm_dtype=None) -> bass.Bass:
    """Build the per-core Bass program for token capacity `cap` (even)."""
    fp32 = mybir.dt.float32
    mmdt = MM_DTYPE if mm_dtype is None else mm_dtype
    nc = bass.Bass("TRN2", target_bir_lowering=False, debug=False,
                   num_devices=N_CORES)

    slab_w = cap + 1024
    # slab k = [x_k | W1[k-block, m0..7]]; slab0 carries 24 extra fp16 cols
    # (16 = b1 per-partition, 8 pad) right after its W1 block.
    slab_d = nc.dram_tensor("slabs", [128, KH * slab_w + 24], mmdt,
                            kind="ExternalInput").ap()
    # W1 strips m8..15, each [128, KH*128] k-major
    w1s_d = nc.dram_tensor("w1s", [128, 8 * KH * 128], mmdt,
                           kind="ExternalInput").ap()
    # W2 k-slabs: col k*HIDDEN + m*128 -> W2[k-block rows, m-block cols]
    w2s_d = nc.dram_tensor("w2s", [128, KF * HIDDEN], mmdt,
                           kind="ExternalInput").ap()
    b2_d = nc.dram_tensor("b2", [128, KH], fp32, kind="ExternalInput").ap()
    yT = nc.dram_tensor("yT", [HIDDEN, cap], mmdt, kind="ExternalOutput").ap()

    with tile.TileContext(nc) as tc:
        with (
            tc.tile_pool(name="xt_pool", bufs=KH) as xt_pool,
            tc.tile_pool(name="w1_pool", bufs=8) as w1_pool,
            tc.tile_pool(name="w2_pool", bufs=1) as w2_pool,
            tc.tile_pool(name="bias_pool", bufs=1) as bias_pool,
            tc.tile_pool(name="ht_pool", bufs=KF) as ht_pool,
            tc.tile_pool(name="out_pool", bufs=4) as out_pool,
            tc.tile_pool(name="ps_pool", bufs=8, space="PSUM") as ps_pool,
        ):
            # ---- input streaming: slabs on the SP queue, in order ----
            slabs = []
            for k in range(KH):
                w = slab_w + (24 if k == 0 else 0)
                off = 0 if k == 0 else k * slab_w + 24
                t = xt_pool.tile([128, w], mmdt, name=f"slab{k}",
                                 tag=f"slab{k}", bufs=1)
                nc.sync.dma_start(t[:], slab_d[:, off:off + w])
                slabs.append(t)

            def x_k(k):
                return slabs[k][:, :cap]

            def b1c(m):
                return slabs[0][:, slab_w + m:slab_w + m + 1]

            b2s = bias_pool.tile([128, KH], fp32, name="b2s", tag="b2s")

            def b2c(m):
                return b2s[:, m:m + 1]

            # ---- PE p-state warmup: pin the ramp start early ----
            wz = bias_pool.tile([128, 1], mmdt, name="wz", tag="wz")
            nc.vector.memset(wz[:], 0.0)
            wps = ps_pool.tile([1, 8], fp32, name="wps", tag="ps")
            for _ in range(2):
                nc.tensor.matmul(wps[0:1, 0:1], wz[:, 0:1], wz[:, 0:1],
                                 start=True, stop=True)

            # ---- phase 1, first half (m0..7): k-major sweeps over slabs ----
            hts = [None] * KF
            ps1 = [ps_pool.tile([128, cap], fp32, name=f"ps1_{m}", tag="ps")
                   for m in range(8)]
            for k in range(KH):
                for m in range(8):
                    nc.tensor.matmul(
                        ps1[m][:],
                        slabs[k][:, cap + m * 128:cap + (m + 1) * 128],
                        x_k(k), start=(k == 0), stop=(k == KH - 1))
                    if k == KH - 1:
                        ht = ht_pool.tile([128, cap], mmdt, name=f"ht{m}",
                                          tag="ht")
                        nc.scalar.activation(
                            ht[:], ps1[m][:],
                            mybir.ActivationFunctionType.Gelu,
                            bias=b1c(m))
                        hts[m] = ht

            # ---- phase 1, second half (m8..15): m-chains on strips ----
            strip_t = {}
            for m in range(8, KF):
                t = w1_pool.tile([128, KH * 128], mmdt, name=f"w1m{m}",
                                 tag="w1s")
                off = (m - 8) * KH * 128
                nc.sync.dma_start(t[:], w1s_d[:, off:off + KH * 128])
                strip_t[m] = t
            for m in range(8, KF):
                psb = ps_pool.tile([128, cap], fp32, name=f"ps1_{m}",
                                   tag="ps")
                for k in range(KH):
                    nc.tensor.matmul(
                        psb[:], strip_t[m][:, k * 128:(k + 1) * 128],
                        x_k(k), start=(k == 0), stop=(k == KH - 1))
                ht = ht_pool.tile([128, cap], mmdt, name=f"ht{m}", tag="ht")
                nc.scalar.activation(
                    ht[:], psb[:],
                    mybir.ActivationFunctionType.Gelu,
                    bias=b1c(m))
                hts[m] = ht

            # ---- phase 2: W2 k-slabs stream behind the strips ----
            w2ks = []
            for k in range(KF):
                t = w2_pool.tile([128, HIDDEN], mmdt, name=f"w2k{k}",
                                 tag=f"w2k{k}", bufs=1)
                nc.sync.dma_start(
                    t[:], w2s_d[:, k * HIDDEN:(k + 1) * HIDDEN])
                w2ks.append(t)
            # b2 (fp32, for the DVE bias-adds): issued after all W2 slabs so
            # its descgen/transfer slot follows them, landing ~2us before the
            # first evacuation needs it.
            nc.sync.dma_start(b2s[:], b2_d[:])

            def w2_lhsT(k, m):
                return w2ks[k][:, m * 128:(m + 1) * 128]

            # k-major sweeps for k < KBREAK across all 8 m-chains
            ps2 = [ps_pool.tile([128, cap], fp32, name=f"ps2_{m}", tag="ps")
                   for m in range(KH)]
            for k in range(KBREAK):
                for m in range(KH):
                    nc.tensor.matmul(
                        ps2[m][:], w2_lhsT(k, m), hts[k][:],
                        start=(k == 0), stop=False)

            # finish chains m-major so completions spread 912ns apart and the
            # bias-add + store pipelines hide behind remaining compute
            ot = None
            for m in range(KH):
                for k in range(KBREAK, KF):
                    nc.tensor.matmul(
                        ps2[m][:], w2_lhsT(k, m), hts[k][:],
                        start=False, stop=(k == KF - 1))
                if m < KH - 2:
                    # paired stores: DVE evacuates even m, ACT odd m, one DMA
                    if m % 2 == 0:
                        ot = out_pool.tile([128, 2 * cap], mmdt,
                                           name=f"ot{m}", tag="ot")
                        nc.vector.tensor_scalar_add(
                            ot[:, :cap], ps2[m][:], b2c(m))
                    else:
                        nc.scalar.activation(
                            ot[:, cap:], ps2[m][:],
                            mybir.ActivationFunctionType.Identity,
                            bias=b2c(m))
                        eng = nc.scalar if (m // 2) % 2 == 0 else nc.sync
                        eng.dma_start(
                            yT[(m - 1) * 128:(m + 1) * 128, :]
                            .rearrange("(c p) t -> p c t", p=128),
                            ot.rearrange("p (c t) -> p c t", c=2))
                else:
                    # final chains: single-m stores, shortest possible tail
                    os_ = out_pool.tile([128, cap], mmdt,
                                        name=f"os{m}", tag=f"os{m % 2}")
                    nc.vector.tensor_scalar_add(
                        os_[:], ps2[m][:], b2c(m))
                    eng = nc.scalar if m % 2 == 0 else nc.sync
                    eng.dma_start(yT[m * 128:(m + 1) * 128, :], os_[:])

    _split_multi_waits(nc)
    return nc


def _get_nc(cap: int) -> bass.Bass:
    key = (cap, MM_DTYPE)
    if key not in _compiled_cache:
        _compiled_cache[key] = _build_nc(cap, MM_DTYPE)
    return _compiled_cache[key]


def _reference_numpy(x, idx, W1, b1, W2, b2):
    """Exact CPU path (erf-gelu in float64). Used only if the device path
    fails — slow but correct."""
    import math
    erf = np.vectorize(math.erf, otypes=[np.float64])
    out = np.zeros_like(x, dtype=np.float64)
    for e in range(NUM_EXPERTS):
        rows = np.nonzero(idx == e)[0]
        if rows.size == 0:
            continue
        h = x[rows].astype(np.float64) @ W1[e].astype(np.float64) + b1[e]
        h = h * 0.5 * (1.0 + erf(h / np.sqrt(2.0)))
        out[rows] = h @ W2[e].astype(np.float64) + b2[e]
    return out.astype(np.float32)


def kernel(x, expert_indices, W1, b1, W2, b2):
    x = np.ascontiguousarray(np.asarray(x, dtype=np.float32))
    idx = np.asarray(expert_indices).astype(np.int64)
    W1 = np.asarray(W1, dtype=np.float32)
    W2 = np.asarray(W2, dtype=np.float32)
    b1 = np.asarray(b1, dtype=np.float32)
    b2 = np.asarray(b2, dtype=np.float32)

    counts = np.bincount(idx, minlength=NUM_EXPERTS)
    cap = max(256, int(-(-int(counts.max()) // 2)) * 2)
    if cap > 512:  # pathological routing, exceeds one PSUM bank
        return _reference_numpy(x, idx, W1, b1, W2, b2)
    nc = _get_nc(cap)

    # dispatch: stable sort tokens by expert
    order = np.argsort(idx, kind="stable")
    starts = np.zeros(NUM_EXPERTS + 1, dtype=np.int64)
    np.cumsum(counts, out=starts[1:])

    np_mmdt = np.float16 if MM_DTYPE == mybir.dt.float16 else np.float32
    slab_w = cap + 1024
    in_maps = []
    tok_of_core = []
    for e in range(NUM_EXPERTS):
        toks = order[starts[e]:starts[e + 1]]
        tok_of_core.append(toks)
        xs = np.zeros((KH, 128, cap), dtype=np_mmdt)
        xs.reshape(HIDDEN, cap)[:, :len(toks)] = x[toks].T
        w1e = W1[e].astype(np_mmdt)          # [H, F]
        w1k = w1e.reshape(KH, 128, FFN)      # k-blocks of rows
        # slabs: [x_0 | W1_0,m0..7] (+ b1/pad on slab0) | [x_1 | W1_1,..] ...
        b1_cols = np.zeros((128, 24), dtype=np_mmdt)
        b1_cols[:, :KF] = b1[e].reshape(KF, 128).T.astype(np_mmdt)
        parts = []
        for k in range(KH):
            parts.append(xs[k])
            parts.append(w1k[k, :, :1024])
            if k == 0:
                parts.append(b1_cols)
        slabs = np.concatenate(parts, axis=1)
        # W1 strips m8..15, each k-major [128, KH*128]
        w1s = (w1e.reshape(KH, 128, KF, 128)[:, :, 8:]
               .transpose(2, 1, 0, 3)       # [m(8), p, k, j]
               .reshape(8, 128, KH * 128)
               .transpose(1, 0, 2).reshape(128, -1))
        # W2 k-slabs: [128, k*HIDDEN + m*128] -> W2[k-block, m-block]
        w2s = (W2[e].astype(np_mmdt).reshape(KF, 128, HIDDEN)
               .transpose(1, 0, 2).reshape(128, -1))
        in_maps.append({
            "slabs": np.ascontiguousarray(slabs),
            "w1s": np.ascontiguousarray(w1s),
            "w2s": np.ascontiguousarray(w2s),
            "b2": np.ascontiguousarray(b2[e].reshape(KH, 128).T),
        })

    try:
        res = _run_spmd_cached(nc, in_maps)
    except Exception:
        try:  # transient failures recover on retry; fall back to the shim
            res = run_bass_kernel_spmd(nc, in_maps,
                                       core_ids=list(range(N_CORES)))
        except Exception:
            return _reference_numpy(x, idx, W1, b1, W2, b2)
    global LAST_RESULTS
    LAST_RESULTS = res

    out = np.zeros((TOKENS, HIDDEN), dtype=np.float32)
    for e in range(NUM_EXPERTS):
        toks = tok_of_core[e]
        out[toks] = res.results[e]["yT"][:, :len(toks)].T.astype(np.float32)
    return out


# revision 29
# speedup vs baseline: 1.1086x; 1.0561x over previous
"""MoE expert-parallel kernel for Trainium2 (8 NeuronCores).

Problem: nn_DistributedExpertPool — each of 2048 tokens (H=1024) is routed to
one of 8 experts; expert e applies Linear(H->F=2048) -> exact GELU ->
Linear(F->H).

Strategy (expert parallelism, matching the sharding hint):
  - Host: sort tokens by expert assignment ("dispatch"), pad each expert's
    token batch to a common capacity CAP (multiple of 2), and pre-swizzle all
    streaming operands into per-partition contiguous images so every device
    DMA is a plain 2-D slice copy.
  - Core c gets expert c's weights plus its token batch and computes
    y.T = W2.T @ gelu(W1.T @ x.T + b1) + b2 entirely on-chip.
  - Host: scatter each core's outputs back to the original token order
    ("combine").

Schedule (everything is paced by the single 360 GB/s DMA pipe, so the stream
order is the design):
  - Phase 1 runs its first 8 F-columns K-MAJOR over eight "slabs"
    [x_k | W1_k,m0..7] whose ~920ns delivery cadence exactly matches the
    912ns PE sweep, so the PE starts ~4.2us in (DMA preamble + first slab +
    semaphore latency is the hard floor) and never starves. b1 rides in
    slab0 as 16 fp16 columns. The remaining 8 F-columns run as m-chains on
    256KB strips (728ns delivery vs 912ns consumption).
  - Phase 2 streams W2 as 16 k-slabs behind the strips and runs k-major
    sweeps for k=0..7 (PSUM-bank bound: exactly 8 accumulators), then
    finishes each m-chain with k=8..15 so chains complete 912ns apart and
    their bias-add + store pipelines hide behind compute; only the last
    chain's evacuate/store/semaphore tail (~3.7us, dominated by fixed DGE
    descgen 625 + trigger delay 650 + completion semaphore 900ns) is
    exposed.
  - Stores are fp16 (host upcasts); output rel-err stays ~5e-4.
  - Two 1-row warmup matmuls pin the PE p-state ramp start early; the first
    two real matmuls still pay the mid-p-state rate, everything after runs
    at full clock.

Matmul operands stream as fp16 (weights are ~N(0, 0.02), activations O(1) —
well inside fp16 range). PSUM accumulation stays fp32. Measured ~5e-4
relative error end-to-end vs the fp32 reference.
"""

import os as _os
import re as _re
import sys as _sys

import numpy as np

try:
    import concourse.bass as bass
except ImportError:  # fresh dirs without the site hook on sys.path
    for _p in ("/opt/trn_rl_repo", "/root/.axon_site/_ro/trn_rl_repo"):
        if _p not in _sys.path:
            _sys.path.append(_p)
    import concourse.bass as bass  # noqa: E402
import concourse.tile as tile
from concourse import mybir
from concourse.bass_utils import run_bass_kernel_spmd  # noqa: F401 (fallback)

_jit_cache: dict[int, tuple] = {}


def _run_spmd_cached(nc, in_maps):
    """run_bass_kernel_spmd's axon/PJRT path with the jitted executable cached
    per program — the concourse shim rebuilds its jax.jit closure every call,
    paying ~1.5s of retrace; reusing one function object makes repeat calls
    dispatch in milliseconds."""
    import jax
    import numpy as _np
    from jax.sharding import Mesh, PartitionSpec
    from jax.experimental.shard_map import shard_map
    from concourse import bass2jax, mybir as _mb

    key = id(nc)
    if key not in _jit_cache:
        bass2jax.install_neuronx_cc_hook()
        partition_name = (nc.partition_id_tensor.name
                          if nc.partition_id_tensor else None)
        in_names, out_names, out_avals = [], [], []
        for alloc in nc.m.functions[0].allocations:
            if not isinstance(alloc, _mb.MemoryLocationSet):
                continue
            name = alloc.memorylocations[0].name
            if alloc.kind == "ExternalInput":
                if name != partition_name:
                    in_names.append(name)
            elif alloc.kind == "ExternalOutput":
                out_names.append(name)
                out_avals.append(jax.core.ShapedArray(
                    tuple(alloc.tensor_shape), _mb.dt.np(alloc.dtype)))
        n_params = len(in_names)
        all_names = list(in_names) + list(out_names)
        if partition_name is not None:
            all_names.append(partition_name)

        def _body(*args):
            operands = list(args)
            if partition_name is not None:
                operands.append(bass2jax.partition_id_tensor())
            return tuple(bass2jax._bass_exec_p.bind(
                *operands, out_avals=tuple(out_avals),
                in_names=tuple(all_names), out_names=tuple(out_names),
                lowering_input_output_aliases=(),
                sim_require_finite=True, sim_require_nnan=True, nc=nc))

        devices = jax.devices()[:N_CORES]
        mesh = Mesh(_np.asarray(devices), ("core",))
        n_outs = len(out_names)
        sharded = jax.jit(
            shard_map(_body, mesh=mesh,
                      in_specs=(PartitionSpec("core"),) * (n_params + n_outs),
                      out_specs=(PartitionSpec("core"),) * n_outs,
                      check_rep=False),
            donate_argnums=tuple(range(n_params, n_params + n_outs)),
            keep_unused=True)
        _jit_cache[key] = (sharded, in_names, out_names, out_avals, n_params)

    sharded, in_names, out_names, out_avals, n_params = _jit_cache[key]
    concat_in = [
        _np.concatenate([_np.asarray(m[name]) for m in in_maps], axis=0)
        for name in in_names]
    concat_zeros = [
        _np.zeros((N_CORES * a.shape[0], *a.shape[1:]), a.dtype)
        for a in out_avals]
    out_arrs = sharded(*concat_in, *concat_zeros)

    class _R:
        results = [
            {name: _np.asarray(out_arrs[i]).reshape(
                N_CORES, *out_avals[i].shape)[c]
             for i, name in enumerate(out_names)}
            for c in range(N_CORES)]
    return _R()

TOKENS = 2048
HIDDEN = 1024
FFN = 2048
NUM_EXPERTS = 8
N_CORES = 8

KH = HIDDEN // 128  # 8 K-tiles for the first matmul
KF = FFN // 128     # 16 K-tiles for the second matmul
KBREAK = 8          # phase 2: k-major sweeps below, m-chains above

_compiled_cache: dict[tuple, bass.Bass] = {}

MM_DTYPE = {"fp32": mybir.dt.float32, "fp32r": mybir.dt.float32r,
            "fp16": mybir.dt.float16}[_os.environ.get("KM_MMDT", "fp16")]


def _split_multi_waits(nc: bass.Bass) -> None:
    """Walrus in this toolchain accepts at most ONE sync-wait per instruction
    ("Too many sync wait commands" in setupSyncWait otherwise). Tile's
    scheduler happily attaches several. Split the extras into NoOps placed
    just before the instruction on the same engine queue — the NX sequencer
    processes them in order, so the semantics are identical."""
    for fn in nc.m.functions:
        for blk in fn.blocks:
            out = []
            changed = False
            for inst in blk.instructions:
                si = inst.sync_info
                if si is not None and si.on_wait is not None and len(si.on_wait) > 1:
                    waits = list(si.on_wait)
                    for j, w in enumerate(waits[:-1]):
                        nop = mybir.InstNoOp(
                            name=f"{inst.name}-wsplit{j}", ins=[], outs=[])
                        nop.engine = inst.engine
                        nop.sync_info = mybir.SyncInfo(on_wait=[w], on_update=[])
                        out.append(nop)
                    inst.sync_info = mybir.SyncInfo(
                        on_wait=[waits[-1]],
                        on_update=list(si.on_update) if si.on_update else [],
                    )
                    changed = True
                out.append(inst)
            if changed:
                blk.instructions = out


def _hoist_prebarrier(nc: bass.Bass) -> None:
    """Move the first sync-queue DMA (slab0), the warmup-zero memset, and the
    warmup matmuls ahead of their queues' entry-barrier instructions in the
    preamble block. Queues execute in order, so the slab0 descriptor-gen and
    the PE p-state ramp start during the barrier rendezvous (~1us) instead of
    after it — the entire delivery line shifts ~0.75us earlier. Safe because
    the hoisted instructions only touch fresh tiles and semaphores that their
    own queue's preamble (still ahead of them, in order) has initialized, and
    the previous launch's exit sequence cleared all semaphores."""
    fn = nc.m.functions[0]
    blocks = fn.blocks
    pre = blocks[0]
    bar_idx = {}
    for j, inst in enumerate(pre.instructions):
        m = _re.match(r"barrier_[A-Za-z]+_(\d+)$", inst.name)
        if m and inst.engine not in bar_idx:
            bar_idx[inst.engine] = j
    if not bar_idx:
        return
    hoists = {mybir.EngineType.SP: [], mybir.EngineType.DVE: [],
              mybir.EngineType.PE: []}
    n_dma = got_ms = 0
    pe_n = 0
    for blk in blocks[1:]:
        keep = []
        for inst in blk.instructions:
            if (n_dma < 2 and isinstance(inst, mybir.InstDMACopy)
                    and inst.engine == mybir.EngineType.SP):
                hoists[mybir.EngineType.SP].append(inst)
                n_dma += 1
                continue
            if (not got_ms and isinstance(inst, mybir.InstMemset)
                    and inst.engine == mybir.EngineType.DVE):
                hoists[mybir.EngineType.DVE].append(inst)
                got_ms = True
                continue
            if (pe_n < 4 and inst.engine == mybir.EngineType.PE
                    and isinstance(inst, (mybir.InstLdweights,
                                          mybir.InstMatmult))):
                hoists[mybir.EngineType.PE].append(inst)
                pe_n += 1
                continue
            keep.append(inst)
        blk.instructions = keep
    out = []
    # All hoisted instructions go to the very front of the preamble — ahead
    # of each queue's RegisterMoves too, which only stage semaphore/register
    # state consumed later (sim and executor both confirm). The memset runs
    # ~100ns in, so the warmup matmuls anchor the PE p-state ramp early
    # enough that its mid-speed window closes before the first real matmul.
    out.extend(hoists.pop(mybir.EngineType.SP, ()))
    out.extend(hoists.pop(mybir.EngineType.DVE, ()))
    out.extend(hoists.pop(mybir.EngineType.PE, ()))
    for j, inst in enumerate(pre.instructions):
        if inst.engine in bar_idx and j == bar_idx[inst.engine]:
            out.extend(hoists.get(inst.engine, ()))
        out.append(inst)
    pre.instructions = out


def _strip_exit2(nc: bass.Bass) -> None:
    """Remove the SECOND exit barrier group (the one after the semaphore
    clears). The first exit barrier already orders all DMA drains before any
    clear; per-queue in-order execution means the next launch's instructions
    cannot run before this launch's clears on the same queue, and cross-queue
    waits are gated by the next launch's entry rendezvous."""
    ids = []
    for fn in nc.m.functions:
        for blk in fn.blocks:
            for inst in blk.instructions:
                m = _re.match(r"barrier_[A-Za-z]+_(\d+)$", inst.name)
                if m:
                    ids.append(int(m.group(1)))
    if len(ids) < 18:  # expect 3 groups x 6
        return
    exit2 = set(sorted(set(ids))[-6:])
    for fn in nc.m.functions:
        for blk in fn.blocks:
            blk.instructions = [
                i for i in blk.instructions
                if not (_re.match(r"barrier_[A-Za-z]+_(\d+)$", i.name)
                        and int(_re.match(r"barrier_[A-Za-z]+_(\d+)$",
                                          i.name).group(1)) in exit2)]


def _overlap_tail_descgen(nc: bass.Bass) -> None:
    """Let the final store's descriptor-gen overlap the final evacuation.
    The store currently waits the DVE bias-add's semaphore; descgen + DGE
    trigger delay (1275ns) only read instruction addresses, so re-keying the
    store's wait to the SAME semaphore the bias-add waits on (the last
    matmul's stop) starts them ~460ns earlier. The actual data read (the
    transfer) still begins >900ns after the bias-add engine-completes, so
    the SBUF read is strictly ordered behind the write in this fixed
    schedule."""
    def ap_names(aps):
        out = set()
        for a in aps or []:
            s = str(a)
            m = _re.search(r"((?:os|ot)\d+)", s)
            if m:
                out.add(m.group(1))
        return out

    evac_by_tile = {}
    for fn in nc.m.functions:
        for blk in fn.blocks:
            for inst in blk.instructions:
                if (inst.engine == mybir.EngineType.DVE
                        and type(inst).__name__.startswith("InstTensorScalar")):
                    for t in ap_names(inst.outs):
                        evac_by_tile[t] = inst
    def sem_tokens(entries):
        toks = set()
        for e in entries or []:
            m = _re.search(r"ant_name[=:] ?['\"]?([A-Za-z0-9_]+)", str(e))
            if m:
                toks.add(m.group(1))
        return toks

    for fn in nc.m.functions:
        for blk in fn.blocks:
            for inst in blk.instructions:
                if not isinstance(inst, mybir.InstDMACopy):
                    continue
                tiles = ap_names(inst.ins)
                ev = next((evac_by_tile[t] for t in tiles
                           if t in evac_by_tile), None)
                if ev is None:
                    continue
                esi = ev.sync_info
                if esi is None or not esi.on_wait:
                    continue
                ssi = inst.sync_info
                orig = list(ssi.on_wait) if ssi and ssi.on_wait else []
                # swap ONLY the wait matching this evac's completion sem;
                # preserve DMA-ring ordering waits etc.
                ev_sems = sem_tokens(esi.on_update)
                kept = [w for w in orig if not (sem_tokens([w]) & ev_sems)]
                if len(kept) == len(orig):
                    continue  # no evac-sem wait present; leave untouched
                inst.sync_info = mybir.SyncInfo(
                    on_wait=kept + list(esi.on_wait),
                    on_update=(list(ssi.on_update)
                               if ssi and ssi.on_update else []))


def _build_nc(cap: int, mm_dtype=None) -> bass.Bass:
    """Build the per-core Bass program for token capacity `cap` (even)."""
    fp32 = mybir.dt.float32
    mmdt = MM_DTYPE if mm_dtype is None else mm_dtype
    nc = bass.Bass("TRN2", target_bir_lowering=False, debug=False,
                   num_devices=N_CORES)

    slab_w = cap + 1024
    # slab k = [x_k | W1[k-block, m0..7]] except: slab0 drops its m7 tile
    # (morphed onto slab1's head) so the critical first transfer is smaller,
    # and carries 24 extra fp16 cols (16 = b1 per-partition, 8 pad) after its
    # W1 block. slab1 = [W1_0,m7 | x_1 | W1_1,m0..7].
    slab_d = nc.dram_tensor("slabs", [128, KH * slab_w + 24], mmdt,
                            kind="ExternalInput").ap()
    s0w = cap + 896 + 24
    s1w = 128 + cap + 1024
    # W1 strips m8..15, each [128, KH*128] k-major
    w1s_d = nc.dram_tensor("w1s", [128, 8 * KH * 128], mmdt,
                           kind="ExternalInput").ap()
    # W2 k-slabs: col k*HIDDEN + m*128 -> W2[k-block rows, m-block cols]
    w2s_d = nc.dram_tensor("w2s", [128, KF * HIDDEN], mmdt,
                           kind="ExternalInput").ap()
    b2_d = nc.dram_tensor("b2", [128, KH], fp32, kind="ExternalInput").ap()
    yT = nc.dram_tensor("yT", [HIDDEN, cap], mmdt, kind="ExternalOutput").ap()

    with tile.TileContext(nc) as tc:
        with (
            tc.tile_pool(name="xt_pool", bufs=KH) as xt_pool,
            tc.tile_pool(name="w1_pool", bufs=8) as w1_pool,
            tc.tile_pool(name="w2_pool", bufs=1) as w2_pool,
            tc.tile_pool(name="bias_pool", bufs=1) as bias_pool,
            tc.tile_pool(name="ht_pool", bufs=KF) as ht_pool,
            tc.tile_pool(name="out_pool", bufs=4) as out_pool,
            tc.tile_pool(name="ps_pool", bufs=8, space="PSUM") as ps_pool,
        ):
            # ---- input streaming: slabs on the SP queue, in order ----
            slabs = []
            off = 0
            for k in range(KH):
                w = s0w if k == 0 else s1w if k == 1 else slab_w
                t = xt_pool.tile([128, w], mmdt, name=f"slab{k}",
                                 tag=f"slab{k}", bufs=1)
                nc.sync.dma_start(t[:], slab_d[:, off:off + w])
                slabs.append(t)
                off += w

            def x_k(k):
                base = 128 if k == 1 else 0
                return slabs[k][:, base:base + cap]

            def w1t(k, m):
                # W1[k-block rows, m-block cols] tile
                if k == 0:
                    if m == 7:
                        return slabs[1][:, 0:128]
                    return slabs[0][:, cap + m * 128:cap + (m + 1) * 128]
                base = 128 + cap if k == 1 else cap
                return slabs[k][:, base + m * 128:base + (m + 1) * 128]

            def b1c(m):
                return slabs[0][:, cap + 896 + m:cap + 896 + m + 1]

            b2s = bias_pool.tile([128, KH], fp32, name="b2s", tag="b2s")

            def b2c(m):
                return b2s[:, m:m + 1]

            # ---- PE p-state warmup: pin the ramp start early ----
            wz = bias_pool.tile([128, 1], mmdt, name="wz", tag="wz")
            nc.vector.memset(wz[:], 0.0)
            wps = ps_pool.tile([1, 8], fp32, name="wps", tag="ps")
            for _ in range(2):
                nc.tensor.matmul(wps[0:1, 0:1], wz[:, 0:1], wz[:, 0:1],
                                 start=True, stop=True)

            # ---- phase 1, first half (m0..7): k-major sweeps over slabs.
            # Sweep 0 covers m0..6 only (slab0 has no m7 tile); m7's k=0
            # matmul opens sweep 1 instead, fed from slab1's head — K
            # accumulation order is free. ----
            hts = [None] * KF
            ps1 = [ps_pool.tile([128, cap], fp32, name=f"ps1_{m}", tag="ps")
                   for m in range(8)]
            for k in range(KH):
                if k == 1:
                    nc.tensor.matmul(ps1[7][:], w1t(0, 7), x_k(0),
                                     start=True, stop=False)
                for m in range(8):
                    if k == 0 and m == 7:
                        continue
                    nc.tensor.matmul(
                        ps1[m][:], w1t(k, m), x_k(k),
                        start=(k == 0), stop=(k == KH - 1))
                    if k == KH - 1:
                        ht = ht_pool.tile([128, cap], mmdt, name=f"ht{m}",
                                          tag="ht")
                        nc.scalar.activation(
                            ht[:], ps1[m][:],
                            mybir.ActivationFunctionType.Gelu,
                            bias=b1c(m))
                        hts[m] = ht

            # ---- phase 1, second half (m8..15): m-chains on strips ----
            strip_t = {}
            for m in range(8, KF):
                t = w1_pool.tile([128, KH * 128], mmdt, name=f"w1m{m}",
                                 tag="w1s")
                off = (m - 8) * KH * 128
                nc.sync.dma_start(t[:], w1s_d[:, off:off + KH * 128])
                strip_t[m] = t
            for m in range(8, KF):
                psb = ps_pool.tile([128, cap], fp32, name=f"ps1_{m}",
                                   tag="ps")
                for k in range(KH):
                    nc.tensor.matmul(
                        psb[:], strip_t[m][:, k * 128:(k + 1) * 128],
                        x_k(k), start=(k == 0), stop=(k == KH - 1))
                ht = ht_pool.tile([128, cap], mmdt, name=f"ht{m}", tag="ht")
                nc.scalar.activation(
                    ht[:], psb[:],
                    mybir.ActivationFunctionType.Gelu,
                    bias=b1c(m))
                hts[m] = ht

            # ---- phase 2: W2 k-slabs stream behind the strips ----
            w2ks = []
            for k in range(KF):
                t = w2_pool.tile([128, HIDDEN], mmdt, name=f"w2k{k}",
                                 tag=f"w2k{k}", bufs=1)
                nc.sync.dma_start(
                    t[:], w2s_d[:, k * HIDDEN:(k + 1) * HIDDEN])
                w2ks.append(t)
            # b2 (fp32, for the DVE bias-adds): issued after all W2 slabs so
            # its descgen/transfer slot follows them, landing ~2us before the
            # first evacuation needs it.
            nc.sync.dma_start(b2s[:], b2_d[:])

            def w2_lhsT(k, m):
                return w2ks[k][:, m * 128:(m + 1) * 128]

            # k-major sweeps for k < KBREAK across all 8 m-chains
            ps2 = [ps_pool.tile([128, cap], fp32, name=f"ps2_{m}", tag="ps")
                   for m in range(KH)]
            for k in range(KBREAK):
                for m in range(KH):
                    nc.tensor.matmul(
                        ps2[m][:], w2_lhsT(k, m), hts[k][:],
                        start=(k == 0), stop=False)

            # finish chains m-major so completions spread 912ns apart and the
            # bias-add + store pipelines hide behind remaining compute
            ot = None
            for m in range(KH):
                for k in range(KBREAK, KF):
                    nc.tensor.matmul(
                        ps2[m][:], w2_lhsT(k, m), hts[k][:],
                        start=False, stop=(k == KF - 1))
                if m < KH - 2:
                    # paired stores: DVE evacuates even m, ACT odd m, one DMA
                    if m % 2 == 0:
                        ot = out_pool.tile([128, 2 * cap], mmdt,
                                           name=f"ot{m}", tag="ot")
                        nc.vector.tensor_scalar_add(
                            ot[:, :cap], ps2[m][:], b2c(m))
                    else:
                        nc.vector.tensor_scalar_add(
                            ot[:, cap:], ps2[m][:], b2c(m))
                        eng = nc.scalar if (m // 2) % 2 == 0 else nc.sync
                        eng.dma_start(
                            yT[(m - 1) * 128:(m + 1) * 128, :]
                            .rearrange("(c p) t -> p c t", p=128),
                            ot.rearrange("p (c t) -> p c t", c=2))
                else:
                    # final chains: single-m stores, shortest possible tail.
                    os_ = out_pool.tile([128, cap], mmdt,
                                        name=f"os{m}", tag=f"os{m % 2}")
                    nc.vector.tensor_scalar_add(
                        os_[:], ps2[m][:], b2c(m))
                    eng = nc.scalar if m % 2 == 0 else nc.sync
                    eng.dma_start(yT[m * 128:(m + 1) * 128, :], os_[:])

    # The epilogue drain's wait list puts the final store's semaphore (the
    # last to fire) second-to-last; rotating the list makes it last so the
    # drain retires the moment it lands instead of processing another NoOp
    # after it. AND-semantics make the order irrelevant for correctness.
    for fn in nc.m.functions:
        for blk in fn.blocks:
            for inst in blk.instructions:
                si = inst.sync_info
                if (si is not None and si.on_wait is not None
                        and len(si.on_wait) >= 8):
                    w = list(si.on_wait)
                    inst.sync_info = mybir.SyncInfo(
                        on_wait=[w[-1]] + w[:-1],
                        on_update=list(si.on_update) if si.on_update else [])
    _overlap_tail_descgen(nc)
    _split_multi_waits(nc)
    _hoist_prebarrier(nc)
    _strip_exit2(nc)
    return nc


def _get_nc(cap: int) -> bass.Bass:
    key = (cap, MM_DTYPE)
    if key not in _compiled_cache:
        _compiled_cache[key] = _build_nc(cap, MM_DTYPE)
    return _compiled_cache[key]


def _reference_numpy(x, idx, W1, b1, W2, b2):
    """Exact CPU path (erf-gelu in float64). Used only if the device path
    fails — slow but correct."""
    import math
    erf = np.vectorize(math.erf, otypes=[np.float64])
    out = np.zeros_like(x, dtype=np.float64)
    for e in range(NUM_EXPERTS):
        rows = np.nonzero(idx == e)[0]
        if rows.size == 0:
            continue
        h = x[rows].astype(np.float64) @ W1[e].astype(np.float64) + b1[e]
        h = h * 0.5 * (1.0 + erf(h / np.sqrt(2.0)))
        out[rows] = h @ W2[e].astype(np.float64) + b2[e]
    return out.astype(np.float32)


def kernel(x, expert_indices, W1, b1, W2, b2):
    x = np.ascontiguousarray(np.asarray(x, dtype=np.float32))
    idx = np.asarray(expert_indices).astype(np.int64)
    W1 = np.asarray(W1, dtype=np.float32)
    W2 = np.asarray(W2, dtype=np.float32)
    b1 = np.asarray(b1, dtype=np.float32)
    b2 = np.asarray(b2, dtype=np.float32)

    counts = np.bincount(idx, minlength=NUM_EXPERTS)
    cap = max(256, int(-(-int(counts.max()) // 2)) * 2)
    if cap > 512:  # pathological routing, exceeds one PSUM bank
        return _reference_numpy(x, idx, W1, b1, W2, b2)
    nc = _get_nc(cap)

    # dispatch: stable sort tokens by expert
    order = np.argsort(idx, kind="stable")
    starts = np.zeros(NUM_EXPERTS + 1, dtype=np.int64)
    np.cumsum(counts, out=starts[1:])

    np_mmdt = np.float16 if MM_DTYPE == mybir.dt.float16 else np.float32
    slab_w = cap + 1024
    in_maps = []
    tok_of_core = []
    for e in range(NUM_EXPERTS):
        toks = order[starts[e]:starts[e + 1]]
        tok_of_core.append(toks)
        xs = np.zeros((KH, 128, cap), dtype=np_mmdt)
        xs.reshape(HIDDEN, cap)[:, :len(toks)] = x[toks].T
        w1e = W1[e].astype(np_mmdt)          # [H, F]
        w1k = w1e.reshape(KH, 128, FFN)      # k-blocks of rows
        # slabs: [x_0 | W1_0,m0..7] (+ b1/pad on slab0) | [x_1 | W1_1,..] ...
        b1_cols = np.zeros((128, 24), dtype=np_mmdt)
        b1_cols[:, :KF] = b1[e].reshape(KF, 128).T.astype(np_mmdt)
        # slab0 = [x_0 | W1_0 m0..6 | b1], slab1 = [W1_0 m7 | x_1 | W1_1],
        # slab k>=2 = [x_k | W1_k m0..7]
        parts = [xs[0], w1k[0, :, :896], b1_cols,
                 w1k[0, :, 896:1024], xs[1], w1k[1, :, :1024]]
        for k in range(2, KH):
            parts.append(xs[k])
            parts.append(w1k[k, :, :1024])
        slabs = np.concatenate(parts, axis=1)
        # W1 strips m8..15, each k-major [128, KH*128]
        w1s = (w1e.reshape(KH, 128, KF, 128)[:, :, 8:]
               .transpose(2, 1, 0, 3)       # [m(8), p, k, j]
               .reshape(8, 128, KH * 128)
               .transpose(1, 0, 2).reshape(128, -1))
        # W2 k-slabs: [128, k*HIDDEN + m*128] -> W2[k-block, m-block]
        w2s = (W2[e].astype(np_mmdt).reshape(KF, 128, HIDDEN)
               .transpose(1, 0, 2).reshape(128, -1))
        in_maps.append({
            "slabs": np.ascontiguousarray(slabs),
            "w1s": np.ascontiguousarray(w1s),
            "w2s": np.ascontiguousarray(w2s),
            "b2": np.ascontiguousarray(b2[e].reshape(KH, 128).T),
        })

    try:
        res = _run_spmd_cached(nc, in_maps)
    except Exception:
        try:  # transient failures recover on retry; fall back to the shim
            res = run_bass_kernel_spmd(nc, in_maps,
                                       core_ids=list(range(N_CORES)))
        except Exception:
            return _reference_numpy(x, idx, W1, b1, W2, b2)
    global LAST_RESULTS
    LAST_RESULTS = res

    out = np.zeros((TOKENS, HIDDEN), dtype=np.float32)
    for e in range(NUM_EXPERTS):
        toks = tok_of_core[e]
        out[toks] = res.results[e]["yT"][:, :len(toks)].T.astype(np.float32)
    return out
